# revision 31
# baseline (speedup 1.0000x reference)
"""Trainium2 Bass kernel for nn_NodeUpdateNetwork (GNN message passing).

Computation per batch b (data-parallel over 8 NeuronCores, one batch each):
    e    = L1-row-normalized masked edge_feat[b]            (host, tiny)
    aggr = e @ node_feat[b]            [N, T, F]
    x    = concat([node_feat, aggr])   [N, T, 2F]
    h    = LeakyReLU(x @ W0.T)         [N, T, 128]
    out  = LeakyReLU(h @ W1.T)         [N, T, 64]

Device dataflow (per t-pair, bf16 matmuls, fp32 PSUM accumulate):
  Stage A : lhsT = nf[:, t:t+2, :] (128x128 stationary); one matmul against
            the concatenated [I | eT] moving operand (FD=256) produces both
            the transposed (channel-major) node features and the aggregated
            neighbors, already transposed - aggregation, concat and layout
            change fused into a single matmul per t-pair.
  Conv0   : two K=64 row-tiled matmul pairs ([W0a;W0a] / [W0b;W0b] stacked
            weights) handle the t-pair-interleaved channel layout; even/odd
            matmuls target disjoint PE row groups and run concurrently.
  Conv1   : weight-swap orientation (h tile stationary, W1T moving) so the
            output lands position-major; no transpose-back needed.
"""

import os
import sys

for _p in ("/opt/trn_rl_repo", "/root/.axon_site/_ro/trn_rl_repo"):
    if os.path.isdir(_p) and _p not in sys.path:
        sys.path.insert(0, _p)

import numpy as np

B, N, T, F = 8, 128, 512, 64
NF = 64
EPS = 1e-12
SLOPE = 0.01

TC = 64            # t-chunk
NCHUNK = T // TC   # 8
NGRP = TC // 8     # 8 groups of 4 t-pairs per chunk

_CACHE = {}


def _build_program():
    import concourse.bacc as bacc
    import concourse.tile as tile
    from concourse import mybir

    f32 = mybir.dt.float32
    bf16 = mybir.dt.bfloat16
    LRELU = mybir.ActivationFunctionType.Lrelu

    nc = bacc.Bacc(None, target_bir_lowering=False)

    nf = nc.dram_tensor("nf", [N, T, F], f32, kind="ExternalInput")
    # ieT = [I | eT] concatenated along columns: [128, 256]
    ieT = nc.dram_tensor("ieT", [N, 2 * N], bf16, kind="ExternalInput")
    # conv0 weights, zero-padded to K=128 so every matmul runs in the
    # uniform 128x128 tile mode (mode switches drain the PE):
    #   w0at=[W0a.T;0] w0ab=[0;W0a.T] w0bt=[W0b.T;0] w0bb=[0;W0b.T]
    w0at = nc.dram_tensor("w0at", [128, 128], bf16, kind="ExternalInput")
    w0ab = nc.dram_tensor("w0ab", [128, 128], bf16, kind="ExternalInput")
    w0bt = nc.dram_tensor("w0bt", [128, 128], bf16, kind="ExternalInput")
    w0bb = nc.dram_tensor("w0bb", [128, 128], bf16, kind="ExternalInput")
    w1t = nc.dram_tensor("w1t", [128, NF], bf16, kind="ExternalInput")
    out = nc.dram_tensor("out", [N, T, NF], f32, kind="ExternalOutput")

    NG = NCHUNK * NGRP  # 64 groups total, software-pipelined

    with tile.TileContext(nc) as tc:
        with (
            tc.tile_pool(name="consts", bufs=1) as consts,
            tc.tile_pool(name="nfc", bufs=10) as nfc_pool,
            tc.tile_pool(name="xsb", bufs=6) as x_pool,
            tc.tile_pool(name="hsb", bufs=8) as h_pool,
            tc.tile_pool(name="osb", bufs=4) as o_pool,
            tc.tile_pool(name="psx", bufs=2, space="PSUM") as psx,   # 1 bank each
            tc.tile_pool(name="psh", bufs=2, space="PSUM") as psh,   # 2 banks each
            tc.tile_pool(name="pso", bufs=2, space="PSUM") as pso,   # 1 bank each
        ):
            ieT_sb = consts.tile([N, 2 * N], bf16)
            w0at_sb = consts.tile([128, 128], bf16)
            w0ab_sb = consts.tile([128, 128], bf16)
            w0bt_sb = consts.tile([128, 128], bf16)
            w0bb_sb = consts.tile([128, 128], bf16)
            w1t_sb = consts.tile([128, NF], bf16)
            nc.sync.dma_start(out=ieT_sb[:], in_=ieT[:])
            nc.sync.dma_start(out=w0at_sb[:], in_=w0at[:])
            nc.sync.dma_start(out=w0ab_sb[:], in_=w0ab[:])
            nc.sync.dma_start(out=w0bt_sb[:], in_=w0bt[:])
            nc.sync.dma_start(out=w0bb_sb[:], in_=w0bb[:])
            nc.sync.dma_start(out=w1t_sb[:], in_=w1t[:])

            nfc_tiles = {}
            state = {}

            def load_group(g):
                # one group = 8 t's; fp32 -> bf16 cast during the load (SWDGE)
                nfg = nfc_pool.tile([N, 8, F], bf16)
                nc.gpsimd.dma_start(out=nfg[:], in_=nf[:, g * 8:(g + 1) * 8, :])
                nfc_tiles[g] = nfg

            def stage_a(g):
                # 4 t-pairs -> interleaved channel-major x (transpose+aggregate)
                nfg = nfc_tiles.pop(g)
                # two 1-bank psum tiles, 2 t-pairs each: [(2t,f), pair, nf/agg, n]
                ps0 = psx.tile([128, 2, 2, 128], f32, tag="psx")
                ps1 = psx.tile([128, 2, 2, 128], f32, tag="psx")
                for j in range(4):
                    lhs = nfg[:, 2 * j:2 * j + 2, :]    # [128, 2, 64] -> M=128
                    ps = (ps0, ps1)[j // 2]
                    nc.tensor.matmul(ps[:, j % 2, :, :], lhs, ieT_sb[:])
                state[('psx', g)] = (ps0, ps1)

            def x_copy(g):
                ps0, ps1 = state.pop(('psx', g))
                # store with nf/agg halves contiguous so conv0 streams a
                # dense rhs; the copy's write AP does the permutation
                x_sb = x_pool.tile([128, 2, 4, 128], bf16)
                dst = x_sb[:].rearrange("p h j n -> p j h n")
                nc.vector.tensor_copy(dst[:, 0:2], ps0[:])
                nc.vector.tensor_copy(dst[:, 2:4], ps1[:])
                state[('x', g)] = x_sb

            def conv0_a(g):
                x_sb = state[('x', g)]
                xn = x_sb[:, 0]  # [128, 4, 128] contiguous
                ps_h = psh.tile([128, 2, 512], f32, tag="psh")
                ps_he = ps_h[:, 0, :]
                ps_ho = ps_h[:, 1, :]
                state[('psh2', g)] = ps_h
                nc.tensor.matmul(ps_he[:], w0at_sb[:], xn[:],
                                 start=True, stop=False)
                nc.tensor.matmul(ps_ho[:], w0ab_sb[:], xn[:],
                                 start=True, stop=False)
                state[('psh', g)] = (ps_he, ps_ho)


            def conv0_b(g):
                x_sb = state.pop(('x', g))
                xa = x_sb[:, 1]
                ps_he, ps_ho = state[('psh', g)]
                nc.tensor.matmul(ps_he[:], w0bt_sb[:], xa[:],
                                 start=False, stop=True)
                nc.tensor.matmul(ps_ho[:], w0bb_sb[:], xa[:],
                                 start=False, stop=True)

            def h_act(g):
                state.pop(('psh', g))
                ps_h = state.pop(('psh2', g))
                h_eo = h_pool.tile([128, 2, 4, 128], bf16)
                nc.scalar.activation(h_eo[:], ps_h[:], LRELU, alpha=SLOPE)
                state[('h', g)] = (h_eo[:, 0], h_eo[:, 1])

            def conv1(g):
                h_e, h_o = state.pop(('h', g))
                ps_o = pso.tile([128, 8, NF], f32)
                for j in range(4):
                    nc.tensor.matmul(ps_o[:, 2 * j, :], h_e[:, j, :], w1t_sb[:])
                    nc.tensor.matmul(ps_o[:, 2 * j + 1, :], h_o[:, j, :], w1t_sb[:])
                state[('pso', g)] = ps_o

            def out_store(g):
                ps_o = state.pop(('pso', g))
                o_sb = o_pool.tile([128, 8, NF], f32)
                if g % 4 == 1:
                    # offload some LReLUs to DVE: PSUM copy + SBUF-legal stt
                    o_raw = o_pool.tile([128, 8, NF], f32, tag="oraw")
                    nc.vector.tensor_copy(o_raw[:], ps_o[:])
                    nc.vector.scalar_tensor_tensor(
                        o_sb[:], o_raw[:], SLOPE, o_raw[:],
                        mybir.AluOpType.mult, mybir.AluOpType.max)
                else:
                    nc.scalar.activation(o_sb[:], ps_o[:], LRELU, alpha=SLOPE)
                t0 = g * 8
                nc.sync.dma_start(out=out[:, t0:t0 + 8, :], in_=o_sb[:])

            # Software pipeline. Per iteration, the PE stream is
            #   c0a(g), sA(g+1), c0b(g), c1(g-1)
            # so stage-A matmuls cover conv0's accumulation drain, and conv1
            # lags one group so its h tiles are already activated.
            PREFETCH = 8
            for g in range(PREFETCH):
                load_group(g)
            stage_a(0)
            x_copy(0)
            for g in range(NG):
                if g + PREFETCH < NG:
                    load_group(g + PREFETCH)
                conv0_a(g)
                if g + 1 < NG:
                    stage_a(g + 1)
                conv0_b(g)
                if g >= 1:
                    conv1(g - 1)
                if g + 1 < NG:
                    x_copy(g + 1)
                h_act(g)
                if g >= 1:
                    out_store(g - 1)
            conv1(NG - 1)
            out_store(NG - 1)

    nc.finalize()
    return nc


def _get_program():
    if "nc" not in _CACHE:
        _CACHE["nc"] = _build_program()
    return _CACHE["nc"]


def _prep_inputs(node_feat, edge_feat, W0, W1):
    import ml_dtypes

    bf = ml_dtypes.bfloat16
    mask = (1.0 - np.eye(N, dtype=np.float32))
    idn = np.eye(N, dtype=np.float32)
    z = np.zeros((F, 128), dtype=np.float32)

    def cat(a, b):
        return np.ascontiguousarray(np.concatenate([a, b], axis=0)).astype(bf)

    w0at = cat(W0[:, :F].T, z)
    w0ab = cat(z, W0[:, :F].T)
    w0bt = cat(W0[:, F:].T, z)
    w0bb = cat(z, W0[:, F:].T)
    w1t = np.ascontiguousarray(W1.T).astype(bf)
    in_maps = []
    for b in range(B):
        e = edge_feat[b] * mask
        e = e / np.maximum(np.abs(e).sum(axis=-1, keepdims=True), EPS)
        ieT = np.concatenate([idn, e.T], axis=1)
        in_maps.append({
            "nf": np.ascontiguousarray(node_feat[b]),
            "ieT": np.ascontiguousarray(ieT).astype(bf),
            "w0at": w0at,
            "w0ab": w0ab,
            "w0bt": w0bt,
            "w0bb": w0bb,
            "w1t": w1t,
        })
    return in_maps


def kernel(node_feat, edge_feat, W0, W1, _trace=False):
    from concourse.bass_utils import run_bass_kernel_spmd

    nc = _get_program()
    in_maps = _prep_inputs(
        np.asarray(node_feat, dtype=np.float32),
        np.asarray(edge_feat, dtype=np.float32),
        np.asarray(W0, dtype=np.float32),
        np.asarray(W1, dtype=np.float32),
    )
    res = run_bass_kernel_spmd(nc, in_maps, list(range(B)), trace=_trace)
    out = np.stack([np.asarray(res.results[i]["out"]) for i in range(B)], axis=0)
    out = out.astype(np.float32)
    if _trace:
        _CACHE["last_result"] = res
    return out


# revision 32
# speedup vs baseline: 1.0708x; 1.0708x over previous
"""Trainium2 Bass kernel for nn_NodeUpdateNetwork (GNN message passing).

Computation per batch b (data-parallel over 8 NeuronCores, one batch each):
    e    = L1-row-normalized masked edge_feat[b]            (host, tiny)
    aggr = e @ node_feat[b]            [N, T, F]
    x    = concat([node_feat, aggr])   [N, T, 2F]
    h    = LeakyReLU(x @ W0.T)         [N, T, 128]
    out  = LeakyReLU(h @ W1.T)         [N, T, 64]

Device dataflow (per t-pair, bf16 matmuls, fp32 PSUM accumulate):
  Stage A : lhsT = nf[:, t:t+2, :] (128x128 stationary); one matmul against
            the concatenated [I | eT] moving operand (FD=256) produces both
            the transposed (channel-major) node features and the aggregated
            neighbors, already transposed - aggregation, concat and layout
            change fused into a single matmul per t-pair.
  Conv0   : two K=64 row-tiled matmul pairs ([W0a;W0a] / [W0b;W0b] stacked
            weights) handle the t-pair-interleaved channel layout; even/odd
            matmuls target disjoint PE row groups and run concurrently.
  Conv1   : weight-swap orientation (h tile stationary, W1T moving) so the
            output lands position-major; no transpose-back needed.
"""

import os
import sys

for _p in ("/opt/trn_rl_repo", "/root/.axon_site/_ro/trn_rl_repo"):
    if os.path.isdir(_p) and _p not in sys.path:
        sys.path.insert(0, _p)

import numpy as np

B, N, T, F = 8, 128, 512, 64
NF = 64
EPS = 1e-12
SLOPE = 0.01

TC = 64            # t-chunk
NCHUNK = T // TC   # 8
NGRP = TC // 8     # 8 groups of 4 t-pairs per chunk

_CACHE = {}


def _build_program():
    import concourse.bacc as bacc
    import concourse.tile as tile
    from concourse import mybir

    f32 = mybir.dt.float32
    bf16 = mybir.dt.bfloat16
    LRELU = mybir.ActivationFunctionType.Lrelu

    nc = bacc.Bacc(None, target_bir_lowering=False)

    nf = nc.dram_tensor("nf", [N, T, F], f32, kind="ExternalInput")
    # ieT = [I | eT] concatenated along columns: [128, 256]
    ieT = nc.dram_tensor("ieT", [N, 2 * N], bf16, kind="ExternalInput")
    # conv0 weights, zero-padded to K=128 so every matmul runs in the
    # uniform 128x128 tile mode (mode switches drain the PE):
    #   w0at=[W0a.T;0] w0ab=[0;W0a.T] w0bt=[W0b.T;0] w0bb=[0;W0b.T]
    w0at = nc.dram_tensor("w0at", [128, 128], bf16, kind="ExternalInput")
    w0ab = nc.dram_tensor("w0ab", [128, 128], bf16, kind="ExternalInput")
    w0bt = nc.dram_tensor("w0bt", [128, 128], bf16, kind="ExternalInput")
    w0bb = nc.dram_tensor("w0bb", [128, 128], bf16, kind="ExternalInput")
    w1t = nc.dram_tensor("w1t", [128, NF], bf16, kind="ExternalInput")
    out = nc.dram_tensor("out", [N, T, NF], f32, kind="ExternalOutput")

    NG = NCHUNK * NGRP  # 64 groups total, software-pipelined

    with tile.TileContext(nc) as tc:
        with (
            tc.tile_pool(name="consts", bufs=1) as consts,
            tc.tile_pool(name="nfc", bufs=10) as nfc_pool,
            tc.tile_pool(name="xsb", bufs=6) as x_pool,
            tc.tile_pool(name="hsb", bufs=8) as h_pool,
            tc.tile_pool(name="osb", bufs=4) as o_pool,
            tc.tile_pool(name="psx", bufs=3, space="PSUM") as psx,   # 1 bank each
            tc.tile_pool(name="psh", bufs=2, space="PSUM") as psh,   # 2 banks each
            tc.tile_pool(name="pso", bufs=1, space="PSUM") as pso,   # 1 bank each
        ):
            ieT_sb = consts.tile([N, 2 * N], bf16)
            w0at_sb = consts.tile([128, 128], bf16)
            w0ab_sb = consts.tile([128, 128], bf16)
            w0bt_sb = consts.tile([128, 128], bf16)
            w0bb_sb = consts.tile([128, 128], bf16)
            w1t_sb = consts.tile([128, NF], bf16)
            nc.sync.dma_start(out=ieT_sb[:], in_=ieT[:])
            nc.sync.dma_start(out=w0at_sb[:], in_=w0at[:])
            nc.sync.dma_start(out=w0ab_sb[:], in_=w0ab[:])
            nc.sync.dma_start(out=w0bt_sb[:], in_=w0bt[:])
            nc.sync.dma_start(out=w0bb_sb[:], in_=w0bb[:])
            nc.sync.dma_start(out=w1t_sb[:], in_=w1t[:])

            nfc_tiles = {}
            state = {}

            def load_group(g):
                # one group = 8 t's; fp32 -> bf16 cast during the load (SWDGE)
                nfg = nfc_pool.tile([N, 8, F], bf16)
                nc.gpsimd.dma_start(out=nfg[:], in_=nf[:, g * 8:(g + 1) * 8, :])
                nfc_tiles[g] = nfg

            def stage_a(g):
                # 4 t-pairs -> interleaved channel-major x (transpose+aggregate)
                nfg = nfc_tiles.pop(g)
                # two 1-bank psum tiles, 2 t-pairs each: [(2t,f), pair, nf/agg, n]
                ps0 = psx.tile([128, 2, 2, 128], f32, tag="psx")
                ps1 = psx.tile([128, 2, 2, 128], f32, tag="psx")
                for j in range(4):
                    lhs = nfg[:, 2 * j:2 * j + 2, :]    # [128, 2, 64] -> M=128
                    ps = (ps0, ps1)[j // 2]
                    nc.tensor.matmul(ps[:, j % 2, :, :], lhs, ieT_sb[:])
                state[('psx', g)] = (ps0, ps1)

            def x_copy(g):
                ps0, ps1 = state.pop(('psx', g))
                # store with nf/agg halves contiguous so conv0 streams a
                # dense rhs; the copy's write AP does the permutation
                x_sb = x_pool.tile([128, 2, 4, 128], bf16)
                dst = x_sb[:].rearrange("p h j n -> p j h n")
                nc.vector.tensor_copy(dst[:, 0:2], ps0[:])
                nc.vector.tensor_copy(dst[:, 2:4], ps1[:])
                state[('x', g)] = x_sb

            def conv0_a(g):
                x_sb = state[('x', g)]
                xn = x_sb[:, 0]  # [128, 4, 128] contiguous
                ps_h = psh.tile([128, 2, 512], f32, tag="psh")
                ps_he = ps_h[:, 0, :]
                ps_ho = ps_h[:, 1, :]
                state[('psh2', g)] = ps_h
                nc.tensor.matmul(ps_he[:], w0at_sb[:], xn[:],
                                 start=True, stop=False)
                nc.tensor.matmul(ps_ho[:], w0ab_sb[:], xn[:],
                                 start=True, stop=False)
                state[('psh', g)] = (ps_he, ps_ho)


            def conv0_b(g):
                x_sb = state.pop(('x', g))
                xa = x_sb[:, 1]
                ps_he, ps_ho = state[('psh', g)]
                nc.tensor.matmul(ps_he[:], w0bt_sb[:], xa[:],
                                 start=False, stop=True)
                nc.tensor.matmul(ps_ho[:], w0bb_sb[:], xa[:],
                                 start=False, stop=True)

            def h_act(g):
                state.pop(('psh', g))
                ps_h = state.pop(('psh2', g))
                h_eo = h_pool.tile([128, 2, 4, 128], bf16)
                nc.scalar.activation(h_eo[:], ps_h[:], LRELU, alpha=SLOPE)
                state[('h', g)] = (h_eo[:, 0], h_eo[:, 1])

            def conv1(g):
                h_e, h_o = state.pop(('h', g))
                ps_o = pso.tile([128, 8, NF], f32)
                for j in range(4):
                    nc.tensor.matmul(ps_o[:, 2 * j, :], h_e[:, j, :], w1t_sb[:])
                    nc.tensor.matmul(ps_o[:, 2 * j + 1, :], h_o[:, j, :], w1t_sb[:])
                state[('pso', g)] = ps_o

            def out_store(g):
                ps_o = state.pop(('pso', g))
                o_sb = o_pool.tile([128, 8, NF], f32)
                if g % 4 == 1:
                    # offload some LReLUs to DVE: PSUM copy + SBUF-legal stt
                    o_raw = o_pool.tile([128, 8, NF], f32, tag="oraw")
                    nc.vector.tensor_copy(o_raw[:], ps_o[:])
                    nc.vector.scalar_tensor_tensor(
                        o_sb[:], o_raw[:], SLOPE, o_raw[:],
                        mybir.AluOpType.mult, mybir.AluOpType.max)
                else:
                    nc.scalar.activation(o_sb[:], ps_o[:], LRELU, alpha=SLOPE)
                t0 = g * 8
                nc.sync.dma_start(out=out[:, t0:t0 + 8, :], in_=o_sb[:])

            # Software pipeline. Per iteration, the PE stream is
            #   c0a(g), sA(g+1), c0b(g), c1(g-1)
            # so stage-A matmuls cover conv0's accumulation drain, and conv1
            # lags one group so its h tiles are already activated.
            PREFETCH = 8
            for g in range(PREFETCH):
                load_group(g)
            stage_a(0)
            x_copy(0)
            for g in range(NG):
                if g + PREFETCH < NG:
                    load_group(g + PREFETCH)
                conv0_a(g)
                if g + 1 < NG:
                    stage_a(g + 1)
                conv0_b(g)
                if g >= 1:
                    conv1(g - 1)
                if g + 1 < NG:
                    x_copy(g + 1)
                h_act(g)
                if g >= 1:
                    out_store(g - 1)
            conv1(NG - 1)
            out_store(NG - 1)

    nc.finalize()
    return nc


def _get_program():
    if "nc" not in _CACHE:
        _CACHE["nc"] = _build_program()
    return _CACHE["nc"]


def _prep_inputs(node_feat, edge_feat, W0, W1):
    import ml_dtypes

    bf = ml_dtypes.bfloat16
    mask = (1.0 - np.eye(N, dtype=np.float32))
    idn = np.eye(N, dtype=np.float32)
    z = np.zeros((F, 128), dtype=np.float32)

    def cat(a, b):
        return np.ascontiguousarray(np.concatenate([a, b], axis=0)).astype(bf)

    w0at = cat(W0[:, :F].T, z)
    w0ab = cat(z, W0[:, :F].T)
    w0bt = cat(W0[:, F:].T, z)
    w0bb = cat(z, W0[:, F:].T)
    w1t = np.ascontiguousarray(W1.T).astype(bf)
    in_maps = []
    for b in range(B):
        e = edge_feat[b] * mask
        e = e / np.maximum(np.abs(e).sum(axis=-1, keepdims=True), EPS)
        ieT = np.concatenate([idn, e.T], axis=1)
        in_maps.append({
            "nf": np.ascontiguousarray(node_feat[b]),
            "ieT": np.ascontiguousarray(ieT).astype(bf),
            "w0at": w0at,
            "w0ab": w0ab,
            "w0bt": w0bt,
            "w0bb": w0bb,
            "w1t": w1t,
        })
    return in_maps


def kernel(node_feat, edge_feat, W0, W1, _trace=False):
    from concourse.bass_utils import run_bass_kernel_spmd

    nc = _get_program()
    in_maps = _prep_inputs(
        np.asarray(node_feat, dtype=np.float32),
        np.asarray(edge_feat, dtype=np.float32),
        np.asarray(W0, dtype=np.float32),
        np.asarray(W1, dtype=np.float32),
    )
    res = run_bass_kernel_spmd(nc, in_maps, list(range(B)), trace=_trace)
    out = np.stack([np.asarray(res.results[i]["out"]) for i in range(B)], axis=0)
    out = out.astype(np.float32)
    if _trace:
        _CACHE["last_result"] = res
    return out
